# revision 15
# baseline (speedup 1.0000x reference)
"""MoE regressor (E=16, H=1024, B=4096, top-2) on 8 trn2 NeuronCores.

Expert-parallel, count-aware schedule: the host computes top-2 routing
(replicated router, fp32), sorts experts by token count, and assigns the
8 largest as phase-0 (one per core) and the 8 smallest as phase-1. Phase
lengths C1/C2 are the max count within each phase group, so per-core
slot count is c(1)+c(9) instead of 2*c(1). Token embeddings are gathered,
transposed to [H, C] and cast to bf16 on the host; weights are cast to
bf16 and laid out as matmul lhsT blocks (partition-major so multi-block
DMAs are contiguous). Each core runs the 2-layer MLP (bf16 matmuls, fp32
PSUM accumulate); the host applies the softmax combine weights and the
output bias (scatter-add) on the returned per-slot outputs.

Device-side scheduling details:
 - DMA triggers cost ~600ns each on the issuing queue, so inputs are
   moved with few, large, descriptor-friendly transfers split across the
   two HWDGE queues (sync + activation).
 - A few warmup matmuls on zeroed SBUF run while the first input DMAs
   land, so the PE DVFS ramp happens in the DMA shadow.
 - ReLU+bias alternates between the scalar and vector engines; the
   second-layer matmul for block m is deferred until after the
   first-layer matmuls of block m+1 so the PE never waits on an
   activation.

Self-contained: hardcodes all shapes.
"""

import numpy as np
import ml_dtypes

import concourse.bass as bass  # noqa: F401
from concourse import bacc
import concourse.mybir as mybir
import concourse.tile as tile
from concourse.bass_utils import run_bass_kernel_spmd

P = 128
B = 4096
H = 1024
E = 16
NCORES = 8
NPH = 2  # phases (experts) per core
NWARM = 4  # PE warmup matmuls

F32 = mybir.dt.float32
BF16 = mybir.dt.bfloat16
BF_NP = ml_dtypes.bfloat16

_CACHE = {}


def _chunks(C):
    """Split C columns into PSUM-bank-sized chunks (<=512 fp32)."""
    out = []
    c0 = 0
    while c0 < C:
        out.append((c0, min(512, C - c0)))
        c0 += 512
    return out


def _build(C1, C2):
    """Per-core kernel: two experts (phase sizes C1 >= C2), 2-layer MLP."""
    nc = bacc.Bacc(None, target_bir_lowering=False)

    ge0 = nc.dram_tensor("ge0", (P, 8, C1), BF16, kind="ExternalInput")
    ge1 = nc.dram_tensor("ge1", (P, 8, C2), BF16, kind="ExternalInput")
    # [kP, phase, m, k, mP] so multi-m slices are contiguous per partition
    w1s = nc.dram_tensor("w1s", (P, NPH, 8, 8, P), BF16, kind="ExternalInput")
    # b1 (cols 0:2, f32) and w2 (cols 2:4, f32; cast to bf16 on device)
    cws = nc.dram_tensor("cws", (P, 8, 2 * NPH), F32, kind="ExternalInput")
    out0 = nc.dram_tensor("out0", (1, C1), F32, kind="ExternalOutput")
    out1 = nc.dram_tensor("out1", (1, C2), F32, kind="ExternalOutput")

    ges = [ge0, ge1]
    outs = [out0, out1]
    Cs = [C1, C2]
    ch1 = _chunks(C1)

    with tile.TileContext(nc) as tc:
        with (
            tc.tile_pool(name="const", bufs=1) as cpool,
            tc.tile_pool(name="ps1", bufs=3, space="PSUM") as ps1_pool,
            tc.tile_pool(name="ps2", bufs=1, space="PSUM") as ps2_pool,
        ):
            cw_sb = cpool.tile([P, 8, 2 * NPH], F32)
            w2_bf = cpool.tile([P, 8, NPH], BF16)
            emb_sb = [
                cpool.tile([P, 8, Cs[ph]], BF16, name=f"emb{ph}", tag=f"emb{ph}")
                for ph in range(NPH)
            ]
            w1_sb = cpool.tile([P, NPH, 8, 8, P], BF16)
            h_sb = [
                cpool.tile([P, 8, Cs[ph]], BF16, name=f"h{ph}", tag=f"h{ph}")
                for ph in range(NPH)
            ]
            o_sb = [
                cpool.tile([1, Cs[ph]], F32, name=f"o{ph}", tag=f"o{ph}")
                for ph in range(NPH)
            ]
            wz = cpool.tile([P, 512], BF16)

            # ---- input DMAs: single queue, strict consumption order ------
            # per-m weight granularity rate-matches the m-loop; FIFO order
            # on one queue means no bandwidth sharing against the critical
            # transfer
            nc.sync.dma_start(w1_sb[:, 0, 0:1], w1s[:, 0, 0:1])
            nc.sync.dma_start(emb_sb[0][:, 0:4], ge0[:, 0:4])
            nc.sync.dma_start(emb_sb[0][:, 4:8], ge0[:, 4:8])
            nc.sync.dma_start(w1_sb[:, 0, 1:2], w1s[:, 0, 1:2])
            nc.sync.dma_start(cw_sb, cws[:, :, :])
            for m in range(2, 8):
                nc.sync.dma_start(w1_sb[:, 0, m:m + 1], w1s[:, 0, m:m + 1])
            nc.sync.dma_start(emb_sb[1][:, :], ge1[:, :])
            for m in range(8):
                nc.sync.dma_start(w1_sb[:, 1, m:m + 1], w1s[:, 1, m:m + 1])

            # ---- PE warmup: absorb the DVFS ramp in the DMA shadow -------
            # (scratch accumulator borrows the p2-0 bank; the first real L2
            # matmul resets it with start=True)
            nc.vector.memset(wz, 0.0)
            pwarm = ps2_pool.tile([P, 512], F32, name="pwarm", tag="p2-0")
            for _ in range(NWARM):
                nc.tensor.matmul(pwarm, wz[:, :P], wz, start=True, stop=True)

            nc.vector.tensor_copy(out=w2_bf, in_=cw_sb[:, :, NPH:])

            for ph in range(NPH):
                C = Cs[ph]
                embT = emb_sb[ph]
                h = h_sb[ph]
                chunks = _chunks(C)
                # full-bank psum tiles (one 2KB bank each, no sub-bank
                # packing that would create false hazards)
                p2 = [
                    ps2_pool.tile([1, 512], F32, name=f"p2_{i}", tag=f"p2-{i}")
                    for i, (c0, cw1) in enumerate(ch1)
                ]

                def l2(j):
                    for i, (c0, cw) in enumerate(chunks):
                        nc.tensor.matmul(
                            p2[i][:, :cw],
                            w2_bf[:, j, ph:ph + 1],
                            h[:, j, c0:c0 + cw],
                            start=(j == 0),
                            stop=(j == 7),
                        )
                        # drain chunk i to SBUF as soon as its group ends,
                        # overlapping the remaining chunks' L2 matmuls
                        if j == 7:
                            nc.vector.tensor_copy(
                                out=o_sb[ph][:, c0:c0 + cw], in_=p2[i][:, :cw]
                            )

                for m in range(8):
                    for i, (c0, cw) in enumerate(chunks):
                        p1 = ps1_pool.tile(
                            [P, 512], F32, name=f"p1_{i}", tag=f"p1-{i}"
                        )
                        for k in range(8):
                            nc.tensor.matmul(
                                p1[:, :cw],
                                w1_sb[:, ph, m, k],
                                embT[:, k, c0:c0 + cw],
                                start=(k == 0),
                                stop=(k == 7),
                            )
                        if m % 2 == 0:
                            nc.scalar.activation(
                                h[:, m, c0:c0 + cw],
                                p1[:, :cw],
                                mybir.ActivationFunctionType.Relu,
                                bias=cw_sb[:, m, ph:ph + 1],
                            )
                        else:
                            nc.vector.tensor_scalar(
                                out=h[:, m, c0:c0 + cw],
                                in0=p1[:, :cw],
                                scalar1=cw_sb[:, m, ph:ph + 1],
                                scalar2=0.0,
                                op0=mybir.AluOpType.add,
                                op1=mybir.AluOpType.max,
                            )
                    # deferred second layer: PE stays on L1(m) while the
                    # activation for m-1 completes
                    if m > 0:
                        l2(m - 1)
                l2(7)

                nc.sync.dma_start(outs[ph][:, :], o_sb[ph])
    nc.finalize()
    return nc


def _route_host(emb, rw, rb):
    logits = emb.astype(np.float32) @ rw.astype(np.float32) + rb.astype(np.float32)
    i1 = np.argmax(logits, axis=1)
    l2m = logits.copy()
    l2m[np.arange(B), i1] = -np.inf
    i2 = np.argmax(l2m, axis=1)
    l1 = logits[np.arange(B), i1]
    l2 = l2m[np.arange(B), i2]
    d = np.exp(l2 - l1)
    wa = (1.0 / (1.0 + d)).astype(np.float32)
    wb = (1.0 - wa).astype(np.float32)
    comb = np.zeros((B, E), np.float32)
    comb[np.arange(B), i1] = wa
    comb[np.arange(B), i2] = wb
    return comb


def kernel(embeddings, router_w, router_b, w1, b1, w2, b2):
    emb = np.ascontiguousarray(np.asarray(embeddings, dtype=np.float32))
    rw = np.asarray(router_w, np.float32)
    rb = np.asarray(router_b, np.float32)
    w1 = np.asarray(w1, np.float32)
    b1 = np.asarray(b1, np.float32)
    w2 = np.asarray(w2, np.float32)
    b2 = np.asarray(b2, np.float32)

    comb = _route_host(emb, rw, rb)
    counts = (comb > 0).sum(axis=0)

    # count-aware schedule: 8 largest experts are phase 0 (one per core),
    # 8 smallest are phase 1; phase length = max count in the phase group.
    ranks = np.argsort(-counts, kind="stable")
    C1 = max(int(counts[ranks[0]]), 1)
    C2 = max(int(counts[ranks[8]]), 1)

    if (C1, C2) not in _CACHE:
        _CACHE[(C1, C2)] = _build(C1, C2)
    nc = _CACHE[(C1, C2)]

    embbf = emb.astype(BF_NP)

    in_maps = []
    toks = []  # per core, per phase: token ids
    for c in range(NCORES):
        es = [int(ranks[c]), int(ranks[8 + c])]
        ctoks = []
        ge_arrs = []
        for ph, e in enumerate(es):
            C = (C1, C2)[ph]
            ids = np.nonzero(comb[:, e] > 0)[0]
            ctoks.append(ids)
            g = np.zeros((C, H), BF_NP)
            g[: len(ids)] = embbf[ids]
            # [C, 8, 128] -> [128(p), 8(hb), C]
            ge_arrs.append(
                np.ascontiguousarray(g.reshape(C, 8, P).transpose(2, 1, 0))
            )
        toks.append(ctoks)
        # [ph, kb, kP, mb, mP] -> [kP, ph, mb, kb, mP]
        w1c = np.ascontiguousarray(
            w1[es].reshape(NPH, 8, P, 8, P).transpose(2, 0, 3, 1, 4).astype(BF_NP)
        )
        cwc = np.empty((P, 8, 2 * NPH), np.float32)
        cwc[:, :, :NPH] = b1[es].reshape(NPH, 8, P).transpose(2, 1, 0)
        cwc[:, :, NPH:] = w2[es, :, 0].reshape(NPH, 8, P).transpose(2, 1, 0)
        in_maps.append({
            "ge0": ge_arrs[0],
            "ge1": ge_arrs[1],
            "w1s": w1c,
            "cws": np.ascontiguousarray(cwc),
        })

    res = run_bass_kernel_spmd(nc, in_maps, core_ids=list(range(NCORES)))

    out = np.zeros((B,), np.float32)
    for c in range(NCORES):
        for ph, e in enumerate([int(ranks[c]), int(ranks[8 + c])]):
            ids = toks[c][ph]
            o = res.results[c][f"out{ph}"][0]
            out[ids] += comb[ids, e] * (o[: len(ids)] + b2[e, 0])
    return out.reshape(B, 1)


# revision 18
# speedup vs baseline: 1.1367x; 1.1367x over previous
"""MoE regressor (E=16, H=1024, B=4096, top-2) on 8 trn2 NeuronCores.

Expert-parallel, count-aware schedule: the host computes top-2 routing
(replicated router, fp32), sorts experts by token count, and assigns the
8 largest as phase-0 (one per core) and the 8 smallest as phase-1. Phase
lengths C1/C2 are the max count within each phase group, so per-core
slot count is c(1)+c(9) instead of 2*c(1). Token embeddings are gathered,
transposed to [H, C] and cast to bf16 on the host; weights are cast to
bf16 and laid out as matmul lhsT blocks. Each core runs the 2-layer MLP
(bf16 L1 matmuls, fp32 PSUM accumulate); the host applies the softmax
combine weights (scatter-add) on the returned per-slot outputs.

The second layer runs mostly on the vector engine: for m-blocks 0..6 the
DVE accumulates partial[k,c] += w2[m*128+k] * h[k,m,c] (fp32, fused
multiply-add), and the PE reduces over partitions with a ones-vector
f32r matmul fused into the same PSUM accumulation group as the final
m-block's direct L2 matmul. This keeps the PE stream almost pure L1.

Self-contained: hardcodes all shapes.
"""

import numpy as np
import ml_dtypes

import concourse.bass as bass  # noqa: F401
from concourse import bacc
import concourse.mybir as mybir
import concourse.tile as tile
from concourse.bass_utils import run_bass_kernel_spmd

P = 128
B = 4096
H = 1024
E = 16
NCORES = 8
NPH = 2  # phases (experts) per core

F32 = mybir.dt.float32
F32R = mybir.dt.float32r
BF16 = mybir.dt.bfloat16
BF_NP = ml_dtypes.bfloat16

_CACHE = {}


def _chunks(C):
    """Split C columns into PSUM-bank-sized chunks (<=512 fp32)."""
    out = []
    c0 = 0
    while c0 < C:
        out.append((c0, min(512, C - c0)))
        c0 += 512
    return out


def _build(C1, C2):
    """Per-core kernel: two experts (phase sizes C1 >= C2), 2-layer MLP."""
    nc = bacc.Bacc(None, target_bir_lowering=False)

    ge0 = nc.dram_tensor("ge0", (P, 8, C1), BF16, kind="ExternalInput")
    ge1 = nc.dram_tensor("ge1", (P, 8, C2), BF16, kind="ExternalInput")
    w1s = nc.dram_tensor("w1s", (NPH, 8, P, 8, P), BF16, kind="ExternalInput")
    b1s = nc.dram_tensor("b1s", (P, 8, NPH), F32, kind="ExternalInput")
    w2s = nc.dram_tensor("w2s", (P, 8, NPH), BF16, kind="ExternalInput")
    w2f = nc.dram_tensor("w2f", (P, 8, NPH), F32, kind="ExternalInput")
    b2s = nc.dram_tensor("b2s", (1, NPH), F32, kind="ExternalInput")
    out0 = nc.dram_tensor("out0", (1, C1), F32, kind="ExternalOutput")
    out1 = nc.dram_tensor("out1", (1, C2), F32, kind="ExternalOutput")

    ges = [ge0, ge1]
    outs = [out0, out1]
    Cs = [C1, C2]
    ch1 = _chunks(C1)

    with tile.TileContext(nc) as tc:
        with (
            tc.tile_pool(name="const", bufs=1) as cpool,
            tc.tile_pool(name="ps1", bufs=2, space="PSUM") as ps1_pool,
            tc.tile_pool(name="ps2", bufs=1, space="PSUM") as ps2_pool,
        ):
            b1_sb = cpool.tile([P, 8, NPH], F32)
            w2_sb = cpool.tile([P, 8, NPH], BF16)
            w2f_sb = cpool.tile([P, 8, NPH], F32)
            b2_sb = cpool.tile([1, NPH], F32)
            ones = cpool.tile([P, 1], BF16)
            emb_sb = [
                cpool.tile([P, 8, Cs[ph]], BF16, name=f"emb{ph}", tag=f"emb{ph}")
                for ph in range(NPH)
            ]
            w1_sb = cpool.tile([P, NPH, 8, 8, P], BF16)
            h_sb = [
                cpool.tile([P, 8, Cs[ph]], BF16, name=f"h{ph}", tag=f"h{ph}")
                for ph in range(NPH)
            ]
            acc_sb = [
                cpool.tile([P, Cs[ph]], BF16, name=f"acc{ph}", tag=f"acc{ph}")
                for ph in range(NPH)
            ]
            o_sb = [
                cpool.tile([1, Cs[ph]], F32, name=f"o{ph}", tag=f"o{ph}")
                for ph in range(NPH)
            ]

            nc.vector.memset(ones, 1.0)

            nc.sync.dma_start(w1_sb[:, 0, 0], w1s[0, 0])
            nc.sync.dma_start(emb_sb[0][:, :4], ge0[:, :4])
            nc.sync.dma_start(emb_sb[0][:, 4:], ge0[:, 4:])
            nc.sync.dma_start(b1_sb, b1s[:, :, :])
            nc.sync.dma_start(w2_sb, w2s[:, :, :])
            nc.sync.dma_start(w2f_sb, w2f[:, :, :])
            nc.sync.dma_start(b2_sb, b2s[:, :])
            for m in range(1, 8):
                nc.sync.dma_start(w1_sb[:, 0, m], w1s[0, m])
            nc.sync.dma_start(emb_sb[1][:, :4], ge1[:, :4])
            nc.sync.dma_start(emb_sb[1][:, 4:], ge1[:, 4:])
            for m in range(8):
                nc.sync.dma_start(w1_sb[:, 1, m], w1s[1, m])

            for ph in range(NPH):
                C = Cs[ph]
                embT = emb_sb[ph]
                h = h_sb[ph]
                acc = acc_sb[ph]
                chunks = _chunks(C)
                p2 = [
                    ps2_pool.tile([1, cw1], F32, name=f"p2_{i}", tag=f"p2-{i}")
                    for i, (c0, cw1) in enumerate(ch1)
                ]

                for m in range(8):
                    for i, (c0, cw) in enumerate(chunks):
                        cw1 = ch1[i][1]
                        p1 = ps1_pool.tile(
                            [P, cw1], F32, name=f"p1_{i}", tag=f"p1-{i}"
                        )
                        for k in range(8):
                            nc.tensor.matmul(
                                p1[:, :cw],
                                w1_sb[:, ph, m, k],
                                embT[:, k, c0:c0 + cw],
                                start=(k == 0),
                                stop=(k == 7),
                            )
                        nc.scalar.activation(
                            h[:, m, c0:c0 + cw],
                            p1[:, :cw],
                            mybir.ActivationFunctionType.Relu,
                            bias=b1_sb[:, m, ph:ph + 1],
                        )
                    # second layer, m-blocks 0..6: DVE partial accumulate
                    if m == 0:
                        nc.vector.tensor_scalar_mul(
                            acc, h[:, 0, :], w2f_sb[:, 0, ph:ph + 1]
                        )
                    elif m < 7:
                        nc.vector.scalar_tensor_tensor(
                            acc,
                            h[:, m, :],
                            w2f_sb[:, m, ph:ph + 1],
                            acc,
                            mybir.AluOpType.mult,
                            mybir.AluOpType.add,
                        )
                # partition-reduce the partials (f32r, full rate at >=256
                # free) fused with the final m-block's direct L2 matmul in
                # one PSUM accumulation group
                for i, (c0, cw) in enumerate(chunks):
                    nc.tensor.matmul(
                        p2[i][:, :cw],
                        ones,
                        acc[:, c0:c0 + cw],
                        start=True,
                        stop=False,
                    )
                for i, (c0, cw) in enumerate(chunks):
                    nc.tensor.matmul(
                        p2[i][:, :cw],
                        w2_sb[:, 7, ph:ph + 1],
                        h[:, 7, c0:c0 + cw],
                        start=False,
                        stop=True,
                    )

                osb = o_sb[ph]
                for i, (c0, cw) in enumerate(chunks):
                    nc.vector.tensor_scalar_add(
                        osb[:, c0:c0 + cw], p2[i][:, :cw], b2_sb[:, ph:ph + 1]
                    )
                nc.sync.dma_start(outs[ph][:, :], osb)
    nc.finalize()
    return nc


def _route_host(emb, rw, rb):
    logits = emb.astype(np.float32) @ rw.astype(np.float32) + rb.astype(np.float32)
    i1 = np.argmax(logits, axis=1)
    l2m = logits.copy()
    l2m[np.arange(B), i1] = -np.inf
    i2 = np.argmax(l2m, axis=1)
    l1 = logits[np.arange(B), i1]
    l2 = l2m[np.arange(B), i2]
    d = np.exp(l2 - l1)
    wa = (1.0 / (1.0 + d)).astype(np.float32)
    wb = (1.0 - wa).astype(np.float32)
    comb = np.zeros((B, E), np.float32)
    comb[np.arange(B), i1] = wa
    comb[np.arange(B), i2] = wb
    return comb


def kernel(embeddings, router_w, router_b, w1, b1, w2, b2):
    emb = np.ascontiguousarray(np.asarray(embeddings, dtype=np.float32))
    rw = np.asarray(router_w, np.float32)
    rb = np.asarray(router_b, np.float32)
    w1 = np.asarray(w1, np.float32)
    b1 = np.asarray(b1, np.float32)
    w2 = np.asarray(w2, np.float32)
    b2 = np.asarray(b2, np.float32)

    comb = _route_host(emb, rw, rb)
    counts = (comb > 0).sum(axis=0)

    # count-aware schedule: 8 largest experts are phase 0 (one per core),
    # 8 smallest are phase 1; phase length = max count in the phase group.
    ranks = np.argsort(-counts, kind="stable")
    C1 = max(int(counts[ranks[0]]), 1)
    C2 = max(int(counts[ranks[8]]), 1)

    if (C1, C2) not in _CACHE:
        _CACHE[(C1, C2)] = _build(C1, C2)
    nc = _CACHE[(C1, C2)]

    embbf = emb.astype(BF_NP)

    in_maps = []
    toks = []  # per core, per phase: token ids
    for c in range(NCORES):
        es = [int(ranks[c]), int(ranks[8 + c])]
        ctoks = []
        ge_arrs = []
        for ph, e in enumerate(es):
            C = (C1, C2)[ph]
            ids = np.nonzero(comb[:, e] > 0)[0]
            ctoks.append(ids)
            g = np.zeros((C, H), BF_NP)
            g[: len(ids)] = embbf[ids]
            # [C, 8, 128] -> [128(p), 8(hb), C]
            ge_arrs.append(
                np.ascontiguousarray(g.reshape(C, 8, P).transpose(2, 1, 0))
            )
        toks.append(ctoks)
        w1c = np.ascontiguousarray(
            w1[es].reshape(NPH, 8, P, 8, P).transpose(0, 3, 2, 1, 4).astype(BF_NP)
        )
        b1c = np.ascontiguousarray(b1[es].reshape(NPH, 8, P).transpose(2, 1, 0))
        w2t = w2[es, :, 0].reshape(NPH, 8, P).transpose(2, 1, 0)
        w2c = np.ascontiguousarray(w2t.astype(BF_NP))
        b2c = np.ascontiguousarray(b2[es, 0].reshape(1, NPH))
        in_maps.append({
            "ge0": ge_arrs[0],
            "ge1": ge_arrs[1],
            "w1s": w1c,
            "b1s": b1c,
            "w2s": w2c,
            "w2f": np.ascontiguousarray(w2t.astype(np.float32)),
            "b2s": b2c,
        })

    res = run_bass_kernel_spmd(nc, in_maps, core_ids=list(range(NCORES)))

    out = np.zeros((B,), np.float32)
    for c in range(NCORES):
        for ph, e in enumerate([int(ranks[c]), int(ranks[8 + c])]):
            ids = toks[c][ph]
            o = res.results[c][f"out{ph}"][0]
            out[ids] += comb[ids, e] * o[: len(ids)]
    return out.reshape(B, 1)
